# revision 56
# baseline (speedup 1.0000x reference)
"""Trainium2 Bass kernel for nn_Net_56650618635135 (gnn_message_passing).

Math (reference):
    edge_value = edge_attr @ Wa[0] + ba            # [E]
    neighbor   = segment_sum(edge_value, edge_index[1], N)   # [N]
    out        = neighbor * Wd + bd                # [N]

Strategy: vertex-cut sharding. Edges are sharded across the 8 cores by
destination-node range (core k owns nodes [k*12500, (k+1)*12500)), so no
all-reduce is needed. Within a core, edges are staged sorted by destination
and packed so each of the 128 SBUF partitions holds a contiguous run of
whole-node segments. The device then:
  1. streams all of edge_attr and computes per-edge v = attr . (Wa*Wd) on the
     tensor engine as bf16 hi/lo pairs (32 edges x 4 channels per moving
     column, 4 accumulating channel-block matmuls per 32-row PE quadrant),
  2. prefix-scans v per partition (vector engine, reading PSUM directly),
  3. gathers the prefix array P at per-node segment-end positions with the
     GPSIMD ap_gather ucode (nodes are length-sorted and dealt in chunks of
     16 so all 16 partitions of a GPSIMD group share identical slot widths,
     satisfying ap_gather's shared-index-per-group semantics),
  4. takes shifted differences and applies the affine tail
     out = dP + (Wd*ba)*seg_len + bd.
The ba term rides on host-shipped segment lengths so zero-padded edges
contribute nothing. bf16 hi/lo keeps relative error ~1e-5 at 3x the fp32
matmul rate; loads are issued as half-round chunks for load/compute overlap.
"""
import sys

sys.path.insert(0, "/opt/trn_rl_repo")

from dataclasses import dataclass

import numpy as np

import concourse.bass as bass
import concourse.bacc as bacc
import concourse.mybir as mybir
from concourse.tile import TileContext

P = 128          # SBUF partitions
EC = 16          # edge channels
NB = 4           # PE output quadrants (32 rows each)
NCB = 4          # channel blocks (4 channels per moving row)
NT = NB * NCB    # rhs tiles per round

f32 = mybir.dt.float32
i32 = mybir.dt.int32


@dataclass(frozen=True)
class Cfg:
    n_nodes: int = 100000
    n_cores: int = 8
    nq: int = 16         # rounds
    f: int = 200         # moving columns per matmul
    cn: int = 112        # gathered positions per partition (slot 0 = zero col)
    dtype: str = "bf16x2"  # "f32" | "bf16x2" for the matmul
    probe: str = ""      # "" | "P" | "G" — debug taps

    @property
    def ce(self):        # v-columns per partition (col 0 reserved zero)
        return self.nq * self.f

    @property
    def nodes_per_core(self):
        return self.n_nodes // self.n_cores


CFG = Cfg()
_CACHE = {}

TRACE = False
LAST_EXEC_NS = None
LAST_PROFILE = None


def build_nc(cfg: Cfg):
    ce, f, nq, cn = cfg.ce, cfg.f, cfg.nq, cfg.cn
    assert cn % 16 == 0
    i16 = mybir.dt.int16
    hilo = cfg.dtype == "bf16x2"
    mmdt = mybir.dt.bfloat16 if hilo else f32
    ncopy = 2 if hilo else 1  # hi/lo copies packed side by side
    nc = bacc.Bacc("TRN2", target_bir_lowering=False)
    rhs = nc.dram_tensor("rhs", [nq, P, ncopy * NT * f], mmdt, kind="ExternalInput")
    lhsT = nc.dram_tensor("lhsT", [P, ncopy * NCB * 32], mmdt, kind="ExternalInput")
    ends = nc.dram_tensor("ends", [P, cn // 16], i16, kind="ExternalInput")
    lens = nc.dram_tensor("lens", [P, cn], f32, kind="ExternalInput")
    consts = nc.dram_tensor("consts", [P, 2], f32, kind="ExternalInput")
    out = nc.dram_tensor("out", [P, cn - 1], f32, kind="ExternalOutput")

    def cast(ap):
        return ap

    with TileContext(nc) as tc:
        with (
            tc.tile_pool(name="const", bufs=1) as cpool,
            tc.tile_pool(name="rhsp", bufs=4) as rpool,
            tc.tile_pool(name="psum", bufs=4, space="PSUM") as ppool,
            tc.tile_pool(name="dpsum", bufs=1, space="PSUM") as dpool,
            tc.tile_pool(name="misc", bufs=1) as mpool,
        ):
            # scratch output for wait-absorbing dummy matmuls (the fused
            # LdWeights+Matmult encoding has a single sync-wait slot, so a
            # cheap PE op absorbs each DMA wait before the real matmuls).
            dmy = dpool.tile([32, 1], f32)

            def absorb(src_tile):
                nc.tensor.matmul(
                    dmy[:],
                    lhsT=src_tile[:, 0:32],
                    rhs=src_tile[:, 0:1],
                    start=True,
                    stop=True,
                    tile_position=(0, 0),
                )
            lt = cpool.tile([P, ncopy * NCB * 32], mmdt)
            nc.scalar.dma_start(out=lt[:], in_=lhsT[:])
            absorb(lt)
            zt = cpool.tile([P, f], f32)
            nc.vector.memset(zt[:], 0.0)
            c_load = cpool.tile([P, 2], f32)
            nc.scalar.dma_start(out=c_load[:], in_=consts[:])
            # DVE-side copy so later tensor_scalar reads have no cross-engine
            # wait (the TensorScalarPtr encoding has a single sync-wait slot).
            c_sb = cpool.tile([P, 2], f32)
            nc.vector.tensor_copy(out=c_sb[:], in_=c_load[:])
            idx_sb = mpool.tile([P, cn // 16], i16)
            nc.scalar.dma_start(out=idx_sb[:], in_=ends[:])
            lens_sb = mpool.tile([P, cn], f32)
            nc.scalar.dma_start(out=lens_sb[:], in_=lens[:])

            # rhs DRAM layout per round: two halves (quadrants b=0,1 | b=2,3),
            # each [P, ncopy*(NT//2)*f]: tiles t'=0..7 then (bf16x2) their lo
            # copies. Half-loads let the first quadrants' matmuls start while
            # the second half is still in flight.
            HT = NT // 2
            rhs_h = rhs.rearrange("q p (h c) -> q h p c", h=2)
            # early-gather split: slots < split_a have segment ends below
            # (split_q+1)*f (host-asserted), so they can be gathered as soon
            # as that prefix region is final, hiding gather latency.
            split_a = 64 if (cn >= 112 and nq * f >= 3200) else 0
            split_q = (12 * 200) // f - 1 if split_a else None
            g_early = (
                mpool.tile([P, split_a], f32, name="g_early") if split_a else None
            )
            p_buf = mpool.tile([P, ce], f32)
            for q in range(nq):
                pt = ppool.tile([P, f], f32)
                for h in range(2):
                    rt = rpool.tile([P, ncopy * HT * f], mmdt)
                    nc.sync.dma_start(out=rt[:], in_=rhs_h[q, h])
                    absorb(rt)
                    for b in (2 * h, 2 * h + 1):
                        for cb in range(NCB):
                            t = b * NCB + cb - h * HT   # tile idx within half
                            # (rhs tile, lhsT block) pairs;
                            # bf16x2: hi*whi + lo*whi + hi*wlo.
                            if hilo:
                                pairs = [
                                    (t * f, 32 * cb),
                                    ((HT + t) * f, 32 * cb),
                                    (t * f, 32 * (NCB + cb)),
                                ]
                            else:
                                pairs = [(t * f, 32 * cb)]
                            for j, (ro, lo_) in enumerate(pairs):
                                nc.tensor.matmul(
                                    pt[32 * b:32 * (b + 1), :],
                                    lhsT=cast(lt[:, lo_:lo_ + 32]),
                                    rhs=cast(rt[:, ro:ro + f]),
                                    start=(cb == 0 and j == 0),
                                    stop=(
                                        cb == NCB - 1 and j == len(pairs) - 1
                                    ),
                                    tile_position=(0, 32 * b),
                                )
                for b in range(NB):
                    rows = slice(32 * b, 32 * (b + 1))
                    initial = (
                        0.0 if q == 0 else p_buf[rows, q * f - 1:q * f]
                    )
                    nc.vector.tensor_tensor_scan(
                        out=p_buf[rows, q * f:(q + 1) * f],
                        data0=pt[rows, :],
                        data1=zt[rows, :],
                        initial=initial,
                        op0=mybir.AluOpType.add,
                        op1=mybir.AluOpType.bypass,
                    )
                if split_a and q == split_q:
                    nc.gpsimd.ap_gather(
                        out_ap=g_early[:],
                        in_ap=p_buf[:, :(split_q + 1) * f],
                        idxs_ap=idx_sb[:, :split_a // 16],
                        channels=P,
                        num_elems=(split_q + 1) * f,
                        d=1,
                        num_idxs=split_a,
                    )
            g_sb = mpool.tile([P, cn], f32)
            if split_a:
                # late gather for the remaining slots (early ones were
                # gathered inside the round loop once their P region final)
                nc.gpsimd.ap_gather(
                    out_ap=g_sb[:, split_a:],
                    in_ap=p_buf[:],
                    idxs_ap=idx_sb[:, split_a // 16:],
                    channels=P,
                    num_elems=ce,
                    d=1,
                    num_idxs=cn - split_a,
                )
            else:
                nc.gpsimd.ap_gather(
                    out_ap=g_sb[:],
                    in_ap=p_buf[:],
                    idxs_ap=idx_sb[:],
                    channels=P,
                    num_elems=ce,
                    d=1,
                    num_idxs=cn,
                )

            if split_a:
                nc.vector.tensor_copy(out=g_sb[:, :split_a], in_=g_early[:])
            d_sb = mpool.tile([P, cn - 1], f32)
            nc.vector.tensor_tensor(
                out=d_sb[:], in0=g_sb[:, 1:], in1=g_sb[:, :cn - 1],
                op=mybir.AluOpType.subtract,
            )
            l_sb = mpool.tile([P, cn - 1], f32)
            nc.vector.tensor_scalar(
                out=l_sb[:], in0=lens_sb[:, 1:],
                scalar1=c_sb[:, 0:1], scalar2=c_sb[:, 1:2],
                op0=mybir.AluOpType.mult, op1=mybir.AluOpType.add,
            )
            o_sb = mpool.tile([P, cn - 1], f32)
            nc.vector.tensor_tensor(
                out=o_sb[:], in0=d_sb[:], in1=l_sb[:],
                op=mybir.AluOpType.add,
            )
            if cfg.probe == "P":
                nc.sync.dma_start(out=out[:], in_=p_buf[:, :cn - 1])
            elif cfg.probe == "G":
                nc.sync.dma_start(out=out[:], in_=g_sb[:, 1:])
            else:
                nc.sync.dma_start(out=out[:], in_=o_sb[:])
    nc.compile()
    return nc


def stage_core(cfg: Cfg, core_attr, core_counts):
    """Stage one core's edges (already sorted by destination, restricted to
    this core's node range) into the device input arrays.

    Nodes are sorted by segment length and dealt in chunks of 16 to
    (group, slot) positions, so all 16 partitions of a GPSIMD group share
    identical slot widths — which makes the segment-end positions uniform
    within each group, as ap_gather requires.

    core_attr:   [Ecore, EC] f32, sorted by destination node
    core_counts: [nodes_per_core] edge counts per node
    Returns (rhs, ends16, lens_arr, node_slot) where node_slot[n] gives the
    flat slot p*(cn-1) + (k-1) in the output tile holding local node n.
    """
    import heapq

    ce, f, nq, cn = cfg.ce, cfg.f, cfg.nq, cfg.cn
    NGRP = P // 16
    n_loc = len(core_counts)
    total = int(core_counts.sum())
    assert total == len(core_attr)

    order = np.argsort(-core_counts, kind="stable")     # by length desc
    n_pad = (-n_loc) % 16
    ids = np.concatenate([order, np.full(n_pad, -1, np.int64)])
    lens_sorted = np.concatenate(
        [core_counts[order], np.zeros(n_pad, core_counts.dtype)]
    )
    chunks = ids.reshape(-1, 16)
    widths = lens_sorted.reshape(-1, 16).max(axis=1).astype(np.int64)
    nchunks = len(widths)
    assert nchunks <= NGRP * (cn - 1), (nchunks, NGRP, cn)

    # LPT: assign chunks (width-desc order) to least-loaded group
    heap = [(0, g) for g in range(NGRP)]
    heapq.heapify(heap)
    grp_slots = [[] for _ in range(NGRP)]               # chunk idx per slot
    chunk_grp = np.empty(nchunks, np.int64)
    chunk_slot = np.empty(nchunks, np.int64)
    for c in range(nchunks):
        load, g = heapq.heappop(heap)
        chunk_grp[c] = g
        chunk_slot[c] = len(grp_slots[g])
        grp_slots[g].append(c)
        heapq.heappush(heap, (load + int(widths[c]), g))
    for g in range(NGRP):
        assert len(grp_slots[g]) <= cn - 1, (g, len(grp_slots[g]))

    # per-group slot start columns (col 0 reserved zero)
    ends16 = np.zeros((P, cn // 16), np.int16)          # wrapped idx tile
    lens_arr = np.zeros((P, cn), np.float32)
    chunk_start = np.empty(nchunks, np.int64)
    for g in range(NGRP):
        ws = widths[grp_slots[g]]
        cum = np.cumsum(ws)
        assert len(cum) == 0 or cum[-1] <= ce - 1, (g, cum[-1] if len(cum) else 0)
        starts = np.concatenate([[1], 1 + cum[:-1]])
        chunk_start[grp_slots[g]] = starts
        ends_list = np.zeros(cn, np.int64)
        ends_list[1:1 + len(cum)] = cum
        if len(cum) < cn - 1:
            ends_list[1 + len(cum):] = cum[-1] if len(cum) else 0
        if cn >= 112 and ce >= 3200:
            # device gathers slots < 64 once the first 2400 P-columns are
            # final (split gather) — their ends must lie below that
            assert ends_list[63] < 2400, (g, ends_list[63])
        for j in range(cn):
            ends16[16 * g + j % 16, j // 16] = ends_list[j]

    # per-node placement
    node_p = np.empty(n_loc, np.int64)
    node_s = np.empty(n_loc, np.int64)
    node_slot = np.empty(n_loc, np.int64)
    cidx = np.repeat(np.arange(nchunks), 16)            # chunk of sorted pos
    lane = np.tile(np.arange(16), nchunks)
    valid = ids >= 0
    nid = ids[valid]
    node_p[nid] = 16 * chunk_grp[cidx[valid]] + lane[valid]
    node_s[nid] = chunk_start[cidx[valid]]
    node_slot[nid] = (
        node_p[nid] * (cn - 1) + (chunk_slot[cidx[valid]] + 1) - 1
    )
    lens_arr[node_p[nid], chunk_slot[cidx[valid]] + 1] = core_counts[nid]

    # scatter edges into [P, ce, EC]
    node_start = np.concatenate([[0], np.cumsum(core_counts)]).astype(np.int64)
    attr_part = np.zeros((P * ce, EC), np.float32)
    if total:
        node_of_e = np.repeat(np.arange(n_loc), core_counts)
        rank = np.arange(total) - node_start[node_of_e]
        dest = node_p[node_of_e] * ce + node_s[node_of_e] + rank
        attr_part[dest] = core_attr
    attr_part = attr_part.reshape(P, ce, EC)

    # rhs staging: tile t = 4*b + cb holds rhs[p = 4*e32 + cc, f_] =
    # attr_part[32*b + e32, q*f + f_, 4*cb + cc]
    A2 = attr_part.reshape(NB, 32, nq, f, NCB, 4)   # [b, e32, q, f_, cb, cc]
    rhs = np.ascontiguousarray(
        A2.transpose(2, 1, 5, 0, 4, 3)               # [q, e32, cc, b, cb, f_]
    ).reshape(nq, P, NT * f)
    if cfg.dtype == "bf16x2":
        import ml_dtypes
        bf16 = ml_dtypes.bfloat16
        rhs4 = rhs.reshape(nq, P, NT, f)
        hi = rhs4.astype(bf16)
        lo = (rhs4 - hi.astype(np.float32)).astype(bf16)
        # per half h: hi tiles 8h..8h+7 then lo tiles 8h..8h+7
        halves = [
            np.concatenate([hi[:, :, 8 * h:8 * h + 8], lo[:, :, 8 * h:8 * h + 8]],
                           axis=2)
            for h in range(2)
        ]
        rhs = np.concatenate(halves, axis=2).reshape(nq, P, 2 * NT * f)
    return rhs, ends16, lens_arr, node_slot


def host_stage(cfg: Cfg, dst, attr, Wa, ba, Wd, bd):
    """Full host staging: returns (in_maps, node_slot_maps)."""
    n_nodes, ncores, npc = cfg.n_nodes, cfg.n_cores, cfg.nodes_per_core
    order = np.argsort(dst, kind="stable")
    attr_s = attr[order]
    counts = np.bincount(dst, minlength=n_nodes).astype(np.int64)
    node_start = np.concatenate([[0], np.cumsum(counts)])

    wa_eff = (np.asarray(Wa, np.float64) * Wd).astype(np.float32)

    # lhsT block cb: [p = 4*e32 + cc, x] = w[4*cb + cc] * (x == e32),
    # shipped as [P, NCB*32] with block cb at cols [32*cb, 32*(cb+1)).
    def build_lhsT(w):
        lt = np.zeros((NCB, P, 32), w.dtype)
        for cb in range(NCB):
            for cc in range(4):
                lt[cb, cc::4, :][np.arange(32), np.arange(32)] = w[4 * cb + cc]
        return np.ascontiguousarray(lt.transpose(1, 0, 2)).reshape(P, NCB * 32)

    if cfg.dtype == "bf16x2":
        import ml_dtypes
        bf16 = ml_dtypes.bfloat16
        w_hi = wa_eff.astype(bf16)
        w_lo = (wa_eff - w_hi.astype(np.float32)).astype(bf16)
        lhsT = np.concatenate([build_lhsT(w_hi), build_lhsT(w_lo)], axis=-1)
    else:
        lhsT = build_lhsT(wa_eff)
    consts = np.broadcast_to(
        np.array([Wd * ba, bd], np.float32), (P, 2)
    ).copy()

    in_maps, slot_maps = [], []
    for k in range(ncores):
        n0, n1 = k * npc, (k + 1) * npc
        e0, e1 = node_start[n0], node_start[n1]
        rhs, ends16, lens_arr, node_slot = stage_core(
            cfg, attr_s[e0:e1], counts[n0:n1]
        )
        in_maps.append({
            "rhs": rhs, "lhsT": lhsT, "ends": ends16, "lens": lens_arr,
            "consts": consts,
        })
        slot_maps.append(node_slot)
    return in_maps, slot_maps


def assemble(cfg: Cfg, results, slot_maps):
    out_full = np.empty(cfg.n_nodes, np.float32)
    npc = cfg.nodes_per_core
    for k in range(cfg.n_cores):
        res = np.asarray(results[k]["out"]).reshape(-1)  # [P*(cn-1)]
        out_full[k * npc:(k + 1) * npc] = res[slot_maps[k]]
    return out_full


def kernel(x, edge_index, edge_attr, Wa, ba, Wd, bd):
    global LAST_EXEC_NS, LAST_PROFILE
    cfg = CFG
    dst = np.asarray(edge_index)[1].astype(np.int32)
    attr = np.ascontiguousarray(np.asarray(edge_attr, dtype=np.float32))
    Wa_ = np.asarray(Wa, np.float32).reshape(-1)
    ba_ = float(np.asarray(ba).reshape(-1)[0])
    Wd_ = float(np.asarray(Wd).reshape(-1)[0])
    bd_ = float(np.asarray(bd).reshape(-1)[0])

    in_maps, slot_maps = host_stage(cfg, dst, attr, Wa_, ba_, Wd_, bd_)

    if cfg not in _CACHE:
        _CACHE[cfg] = build_nc(cfg)
    nc = _CACHE[cfg]

    from concourse.bass_utils import run_bass_kernel_spmd
    res = run_bass_kernel_spmd(
        nc, in_maps, core_ids=list(range(cfg.n_cores)), trace=TRACE
    )
    LAST_EXEC_NS = res.exec_time_ns
    LAST_PROFILE = res.profile_json
    return assemble(cfg, res.results, slot_maps)
